# revision 12
# baseline (speedup 1.0000x reference)
"""Trainium2 Bass kernel for nn_ConsolidationModel.

Mathematical reduction (verified bit-exact against the reference scan):
the scan's control flow is data-independent (count depends only on t).
Consolidation fires at t=15/31/47, but between consecutive firings the
8-slot FIFO receives 4 appends + 12 shift-appends, which evicts every
consolidated row before the next firing — and after the last firing
(t=47) there are 4 appends + 11 shifts, so at t=62 the buffer holds
exactly the embeddings of tokens 55..62 with count=8.  The model output
is therefore:

    mem  = mean_p embed[seqs[:, 55+p]]          (p = 0..7)
    h    = concat([embed[query_tok], mem], -1)  (B, 128)
    out  = relu(h @ r1_w.T + r1_b) @ r2_w.T + r2_b

Device algorithm (per core, batch shard of 256 rows).  Profiling
learnings baked in: (1) SDMA is descriptor-bound at ~180ns/descriptor/
engine (stride-0 broadcast sources are ~2x worse), so ALL input ships
as ONE host-replicated tensor = 128 large descriptors on one queue;
(2) Scalar-engine ACTIVATE and GpSimd custom ops each trigger
background table/library DMAs that park an SDMA engine and straggle
every DMA completion by ~2.3us, so neither engine executes any compute
op; (3) matmuls cost ~165ns + 0.9ns/col, so the histogram is pre-added
on the DVE (bf16 2x) down to 2 accumulated matmuls.

  wide (128, 1480) i16 <- ONE DMA: cols 0:1280 = tokens broadcast
       (rows replicated on host: top half = tail positions 0..3
       position-major + query; bottom half = positions 4..7 offset
       +64, query slot = -1 sentinel), then bit-packed constants:
       iota f32, r1b/r2b f32, [Bm;Bm]/[A;0]/r2w.T bf16 — all read on
       device via bitcast APs.
  m8   = (wide == iota) ONE is_equal tensor_scalar (i16 SBUF, 4x DVE)
  hist = 3 bf16 2x tensor_tensor pre-adds over the 4 position chunks
  hidT = [A;0]^T @ m8_query + [Bm;Bm]^T @ hist  (2 bf16 matmuls,
         K=128 contracts both partition halves = the position sum)
  hid  = relu(hidT + r1b)  fused 2-op DVE tensor_scalar (add, max)
  logT = r2wT^T @ hid ; + r2b — pipelined in batch halves so the two
         32KB output DMAs (sync + scalar queues) issue early.

Sharding: pure data parallel over batch across 8 cores; parameters
replicated.  Host transposes each (64, 256) shard back to (256, 64).
"""

import numpy as np

N_CORES = 8
B = 2048           # full batch
BS = B // N_CORES  # 256 per-core batch shard
H = 64             # hidden dim
V = 64             # vocab
TAIL_LO, TAIL_HI = 55, 63  # token positions that survive in the buffer
NPOS = TAIL_HI - TAIL_LO   # 8

# wide tensor column map (i16 columns)
C_TOK = 0          # 0:1280   tokens (4 position-pair chunks + query)
C_IOTA = 5 * BS          # 1280:1282  iota 0..127 as packed f32
C_R1B = C_IOTA + 2       # 1282:1284  r1_b as packed f32 (rows 0:64)
C_R2B = C_R1B + 2        # 1284:1286  r2_b as packed f32 (rows 0:64)
C_BM = C_R2B + 2         # 1286:1350  [Bm;Bm] bf16 bits
C_A = C_BM + H           # 1350:1414  [A;0] bf16 bits
C_R2W = C_A + H          # 1414:1478  [r2w.T;0] bf16 bits
C_END = C_R2W + H + 2    # 1480 (pad to keep 4B-aligned total)

_compiled_nc = None


def _build_program():
    import concourse.bacc as bacc
    import concourse.mybir as mybir
    from concourse import tile

    f32 = mybir.dt.float32
    bf16 = mybir.dt.bfloat16
    i16 = mybir.dt.int16
    eq = mybir.AluOpType.is_equal
    add = mybir.AluOpType.add
    mx = mybir.AluOpType.max

    nc = bacc.Bacc("TRN2", target_bir_lowering=False, debug=False,
                   num_devices=N_CORES)

    wide_d = nc.declare_dram_parameter("wide", [2 * H, C_END], i16,
                                       isOutput=False)
    out_d = nc.declare_dram_parameter("logT", [V, BS], f32, isOutput=True)

    with tile.TileContext(nc) as tc:
        with (
            tc.tile_pool(name="sb", bufs=1) as pool,
            tc.tile_pool(name="ps", bufs=1, space="PSUM") as pp,
        ):
            # the whole input: one DMA, one semaphore, 128 descriptors
            wide = pool.tile([2 * H, C_END], i16)
            nc.sync.dma_start(wide[:], wide_d[:])

            iota = wide[:, C_IOTA:C_IOTA + 2].bitcast(f32)
            r1b = wide[0:H, C_R1B:C_R1B + 2].bitcast(f32)
            r2b = wide[0:V, C_R2B:C_R2B + 2].bitcast(f32)
            w_bm = wide[:, C_BM:C_BM + H].bitcast(bf16)
            w_a = wide[:, C_A:C_A + H].bitcast(bf16)
            w_r2 = wide[0:H, C_R2W:C_R2W + H].bitcast(bf16)

            # one-hot masks: ONE is_equal over all 8 positions + query
            m8 = pool.tile([2 * V, 5 * BS], bf16)
            nc.vector.tensor_scalar(m8[:], wide[:, 0:5 * BS], iota, None, eq)

            # histogram: 2-level bf16 2x fold over the 4 position chunks
            s2 = pool.tile([2 * V, 2 * BS], bf16)
            nc.vector.tensor_add(s2[:], m8[:, 0:2 * BS], m8[:, 2 * BS:4 * BS])
            hist = pool.tile([2 * V, BS], bf16)
            nc.vector.tensor_add(hist[:], s2[:, 0:BS], s2[:, BS:2 * BS])

            # hidT = [A;0]^T @ q1h + [Bm;Bm]^T @ hist
            hidT_ps = pp.tile([H, BS], f32, tag="hid")
            nc.tensor.matmul(hidT_ps[:], w_a, m8[:, 4 * BS:5 * BS],
                             start=True, stop=False)
            nc.tensor.matmul(hidT_ps[:], w_bm, hist[:],
                             start=False, stop=True)

            # tail pipelined in batch halves: relu -> logT matmul ->
            # +r2b -> DMA out, so the first DMA issues early
            hid = pool.tile([H, BS], bf16)
            logT_ps = pp.tile([V, BS], f32, tag="log")
            logT_sb = pool.tile([V, BS], f32)
            hb = 160  # asymmetric split: shorter last chain -> earlier DMA
            for lo, hi, dma in ((0, hb, nc.sync), (hb, BS, nc.scalar)):
                nc.vector.tensor_scalar(hid[:, lo:hi], hidT_ps[:, lo:hi],
                                        r1b, 0.0, add, mx)
                nc.tensor.matmul(logT_ps[:, lo:hi], w_r2, hid[:, lo:hi],
                                 start=True, stop=True)
                nc.vector.tensor_scalar(logT_sb[:, lo:hi], logT_ps[:, lo:hi],
                                        r2b, None, add)
                dma.dma_start(out_d[:, lo:hi], logT_sb[:, lo:hi])

    nc.compile()
    return nc


def _prep_in_maps(inputs):
    import ml_dtypes
    bft = ml_dtypes.bfloat16

    embed = np.asarray(inputs["embed"], dtype=np.float32)[:V]      # (64, 64)
    r1_w = np.asarray(inputs["r1_w"], dtype=np.float32)            # (64, 128)
    r1_b = np.asarray(inputs["r1_b"], dtype=np.float32)            # (64,)
    r2_w = np.asarray(inputs["r2_w"], dtype=np.float32)            # (64, 64)
    r2_b = np.asarray(inputs["r2_b"], dtype=np.float32)            # (64,)
    seqs = np.asarray(inputs["seqs"])                              # (B, 64) int
    query = np.asarray(inputs["query_tok"])                        # (B,) int

    A = embed @ r1_w[:, :H].T                                      # (64v, 64h)
    Bm = (embed @ r1_w[:, H:].T) * np.float32(1.0 / NPOS)          # (64v, 64h)

    # constant columns (identical for every core), as i16 bit patterns
    cc = np.zeros((2 * H, C_END - C_IOTA), np.uint16)
    iota = np.arange(2 * V, dtype=np.float32)
    cc[:, 0:2] = iota.view(np.uint16).reshape(2 * V, 2)
    cc[0:H, 2:4] = r1_b.view(np.uint16).reshape(H, 2)
    cc[0:V, 4:6] = r2_b.view(np.uint16).reshape(V, 2)
    cc[0:V, 6:6 + H] = Bm.astype(bft).view(np.uint16)
    cc[V:2 * V, 6:6 + H] = cc[0:V, 6:6 + H]
    cc[0:V, 6 + H:6 + 2 * H] = A.astype(bft).view(np.uint16)
    cc[0:H, 6 + 2 * H:6 + 3 * H] = r2_w.T.astype(bft).view(np.uint16)

    # token region: rows replicated on host (no stride-0 descriptors)
    tail = seqs[:, TAIL_LO:TAIL_HI].astype(np.int16)               # (B, 8)
    wide = np.empty((N_CORES, 2 * H, C_END), np.int16)
    wide[:, :, C_IOTA:] = cc.view(np.int16)
    for c in range(N_CORES):
        sh = tail[c * BS:(c + 1) * BS]                             # (256, 8)
        row0 = np.empty(5 * BS, np.int16)
        row0[0:4 * BS] = sh[:, 0:4].T.reshape(4 * BS)
        row0[4 * BS:5 * BS] = query[c * BS:(c + 1) * BS]
        row1 = np.empty(5 * BS, np.int16)
        row1[0:4 * BS] = sh[:, 4:8].T.reshape(4 * BS) + V
        row1[4 * BS:5 * BS] = -1
        wide[c, 0:V, 0:5 * BS] = row0
        wide[c, V:2 * V, 0:5 * BS] = row1
    return [{"wide": wide[c]} for c in range(N_CORES)]


def kernel(**inputs):
    global _compiled_nc
    from concourse.bass_utils import run_bass_kernel_spmd

    in_maps = _prep_in_maps(inputs)
    if _compiled_nc is None:
        _compiled_nc = _build_program()
    res = run_bass_kernel_spmd(_compiled_nc, in_maps, list(range(N_CORES)))
    out = np.empty((B, V), np.float32)
    for c in range(N_CORES):
        out[c * BS:(c + 1) * BS] = res.results[c]["logT"].T
    return out


if __name__ == "__main__":
    rng = np.random.default_rng(0)
    demo = {
        "embed": rng.standard_normal((V + 2, H)).astype(np.float32),
        "r1_w": rng.standard_normal((H, 2 * H)).astype(np.float32) * 0.05,
        "r1_b": rng.standard_normal(H).astype(np.float32) * 0.02,
        "r2_w": rng.standard_normal((V, H)).astype(np.float32) * 0.05,
        "r2_b": rng.standard_normal(V).astype(np.float32) * 0.02,
        "seqs": rng.integers(0, V, (B, 64)),
        "query_tok": rng.integers(0, V, (B,)),
    }
    out = kernel(**demo)
    tail = demo["embed"][demo["seqs"][:, TAIL_LO:TAIL_HI]]
    mem = tail.sum(1) / NPOS
    h = np.concatenate([demo["embed"][demo["query_tok"]], mem], -1)
    exp = np.maximum(h @ demo["r1_w"].T + demo["r1_b"], 0) @ demo["r2_w"].T + demo["r2_b"]
    err = np.abs(out - exp).max() / np.abs(exp).max()
    print("self-check rel err:", err)
